# revision 18
# baseline (speedup 1.0000x reference)
"""GAT (GATConv + BatchNorm + ReLU + Linear) Trainium2 Bass kernel, 8-core SPMD.

v3 design — no device-side random gathers (indirect DMA on this platform is
only reliable in forms too slow for 1.7M edges):
  - Host (integer-only prep): dedup multi-edges into multiplicities, add
    self-loops, partition edges by destination core, sort by dst, pack into
    uniform groups of G_D destination nodes x GT 128-edge tiles, and expand
    x into edge-ordered bf16 streams x[src_e], x[dst_e] (transposed layout,
    zero-padded). Pure indexing/layout: all arithmetic happens on device.
  - Device, per 128-edge tile: two PE matmuls give h_e=W@x_src (with a_src
    folded in as an extra output column) and a_dst_e. e = exp(leakyrelu(
    a_src+a_dst)) (DVE+ACT, batched per group). A dst-equality mask weighted
    by edge multiplicity (one DVE tensor_scalar) and a PE matmul scatter-add
    e*h_e (plus e itself via a ones column) into the group's PSUM
    accumulator over 128 destination slots; softmax denominators fall out as
    PSUM column 64. out = num/den.
  - BN batch stats accumulated on DVE, reduced by one matmul + a tiny
    AllReduce; BN+ReLU fused into one ACT op on the PE-transposed tile; FC
    on PE; rows written back with compile-time DMAs (uniform groups make
    every destination range static).
"""

import math
from contextlib import ExitStack

import numpy as np
import ml_dtypes

import concourse.bacc as bacc
import concourse.mybir as mybir
import concourse.tile as tile

F32 = mybir.dt.float32
BF16 = mybir.dt.bfloat16

NEG_SLOPE = 0.2
BN_EPS = 1e-5
P = 128

BF = ml_dtypes.bfloat16


# ----------------------------------------------------------------- host prep
def _host_prep(edge_index, x, n_nodes, n_cores):
    """Integer-only edge preprocessing + input layout expansion."""
    npc = n_nodes // n_cores

    src = np.asarray(edge_index[0], dtype=np.int64)
    dst = np.asarray(edge_index[1], dtype=np.int64)
    loops = np.arange(n_nodes, dtype=np.int64)
    src = np.concatenate([src, loops])
    dst = np.concatenate([dst, loops])

    # dedup (u,v) multi-edges into integer multiplicities
    key = dst * n_nodes + src
    ukey, mult = np.unique(key, return_counts=True)
    dst = (ukey // n_nodes).astype(np.int64)
    src = (ukey % n_nodes).astype(np.int64)
    # unique() sorts keys, so edges are sorted by dst (then src)

    bounds = np.searchsorted(dst, np.arange(n_cores + 1) * npc)
    deg_all = []
    for c in range(n_cores):
        lo, hi = bounds[c], bounds[c + 1]
        deg_all.append(np.bincount(dst[lo:hi] - c * npc, minlength=npc))

    # choose uniform group geometry: G_D dsts per group, GT tiles per group
    best = None
    for gd in (128, 120, 112, 104, 96, 88, 80, 72, 64):
        ng = (npc + gd - 1) // gd
        maxc = 0
        for c in range(n_cores):
            d = deg_all[c]
            pad = np.zeros(ng * gd - npc, dtype=d.dtype)
            cnt = np.concatenate([d, pad]).reshape(ng, gd).sum(1)
            maxc = max(maxc, int(cnt.max()))
        gt = (maxc + P - 1) // P
        cost = ng * gt
        if best is None or cost < best[0]:
            best = (cost, gd, ng, gt)
    _, gd, ng, gt = best
    gcap = gt * P

    xT = np.ascontiguousarray(np.asarray(x, dtype=np.float32).T)  # [64, N]
    fi = xT.shape[0]
    nt = ng * gt
    per_core = []
    for c in range(n_cores):
        lo, hi = bounds[c], bounds[c + 1]
        s, d, m = src[lo:hi], dst[lo:hi] - c * npc, mult[lo:hi]
        g_of = d // gd
        cnt = np.bincount(g_of, minlength=ng)
        assert cnt.max() <= gcap
        # edge slot within the padded group layout
        off = np.zeros(ng + 1, dtype=np.int64)
        np.cumsum(cnt, out=off[1:])
        pos = g_of * gcap + (np.arange(hi - lo) - off[g_of])
        ne = ng * gcap
        srcs = np.zeros(ne, dtype=np.int64)
        dstl = np.full(ne, -1.0, dtype=np.float32)
        mm = np.zeros(ne, dtype=np.float32)
        valid = np.zeros(ne, dtype=bool)
        srcs[pos] = s
        dstl[pos] = (d - g_of * gd).astype(np.float32)
        mm[pos] = m.astype(np.float32)
        valid[pos] = True
        # edge (g, t, p): stream position g*gcap + t*128 + p
        dglob = np.zeros(ne, dtype=np.int64)
        dglob[pos] = d + c * npc
        xs = np.zeros((fi, ne), dtype=BF)
        xd = np.zeros((fi, ne), dtype=BF)
        xs[:, valid] = xT[:, srcs[valid]].astype(BF)
        xd[:, valid] = xT[:, dglob[valid]].astype(BF)
        per_core.append({
            "xsrc": np.ascontiguousarray(xs),
            "xdst": np.ascontiguousarray(xd),
            "dstl": np.ascontiguousarray(dstl.reshape(nt, P).T),
            "mlt": np.ascontiguousarray(mm.reshape(nt, P).T),
        })
    return per_core, gd, ng, gt, npc


# ------------------------------------------------------------- bass program
def _build_program(n_cores, gd, ng, gt, npc, fi, fh, fo, n_nodes):
    nt = ng * gt
    nc = bacc.Bacc("TRN2", target_bir_lowering=False, debug=False,
                   num_devices=n_cores)

    def inp(name, shape, dtype=F32):
        return nc.dram_tensor(name, shape, dtype, kind="ExternalInput").ap()

    xsrc = inp("xsrc", [fi, nt * P], BF16)
    xdst = inp("xdst", [fi, nt * P], BF16)
    dstl = inp("dstl", [P, nt])
    mlt = inp("mlt", [P, nt])
    Wjk = inp("Wjk", [fh, fi])
    WT = inp("WT", [fi, fh])
    attS = inp("attS", [fh, 1])
    attD = inp("attD", [fh, 1])
    fcWT = inp("fcWT", [fh, fo])
    fcb = inp("fcb", [1, fo])
    gam = inp("gam", [fh, 1])
    bet = inp("bet", [fh, 1])
    out = nc.dram_tensor("out", [npc, fo], F32, kind="ExternalOutput").ap()
    stats_d = nc.dram_tensor("stats_d", [fh, 2], F32).ap()
    rg = [list(range(n_cores))]

    with tile.TileContext(nc) as tc, ExitStack() as ctx:
        cst = ctx.enter_context(tc.tile_pool(name="cst", bufs=1))
        big = ctx.enter_context(tc.tile_pool(name="big", bufs=1))

        # ---- constants
        iota_i = cst.tile([P, P], mybir.dt.int32)
        nc.gpsimd.iota(iota_i[:], pattern=[[1, P]], base=0,
                       channel_multiplier=0)
        iota_b = cst.tile([P, P], BF16)
        nc.vector.tensor_copy(iota_b[:], iota_i[:])
        ident = cst.tile([P, P], F32)
        from concourse.masks import make_identity
        make_identity(nc, ident[:])
        ones_col = cst.tile([P, 1], F32)
        nc.vector.memset(ones_col[:], 1.0)
        ones_row = cst.tile([1, P], F32)
        nc.vector.memset(ones_row[:], 1.0)
        eps_den = cst.tile([P, 1], F32)
        nc.vector.memset(eps_den[:], 1e-30)
        eps_bn = cst.tile([fh, 1], F32)
        nc.vector.memset(eps_bn[:], BN_EPS)

        Wjk_sb = cst.tile([fh, fi], F32)
        nc.sync.dma_start(Wjk_sb[:], Wjk)
        attS_sb = cst.tile([fh, 1], F32)
        nc.sync.dma_start(attS_sb[:], attS)
        attD_sb = cst.tile([fh, 1], F32)
        nc.sync.dma_start(attD_sb[:], attD)
        fcWT_sb = cst.tile([fh, fo], F32)
        nc.sync.dma_start(fcWT_sb[:], fcWT)
        fcb_sb = cst.tile([1, fo], F32)
        nc.sync.dma_start(fcb_sb[:], fcb)
        gam_sb = cst.tile([fh, 1], F32)
        nc.sync.dma_start(gam_sb[:], gam)
        bet_sb = cst.tile([fh, 1], F32)
        nc.sync.dma_start(bet_sb[:], bet)

        # Wext (bf16): [W.T | a_src coef]; coefD separate
        WT_sb = cst.tile([fi, fh], F32)
        nc.sync.dma_start(WT_sb[:], WT)
        Wext = cst.tile([fi, fh + 1], BF16)
        coefD = cst.tile([fi, 1], BF16)
        fcb_bc = cst.tile([P, fo], F32)
        with tc.tile_pool(name="pcst", bufs=1, space="PSUM") as pcst:
            nc.scalar.copy(Wext[:, 0:fh], WT_sb[:])
            cps = pcst.tile([fh, 2], F32)
            nc.tensor.matmul(out=cps[:, 0:1], lhsT=Wjk_sb[:], rhs=attS_sb[:],
                             start=True, stop=True)
            nc.tensor.matmul(out=cps[:, 1:2], lhsT=Wjk_sb[:], rhs=attD_sb[:],
                             start=True, stop=True)
            nc.vector.tensor_copy(Wext[:, fh:fh + 1], cps[:, 0:1])
            nc.vector.tensor_copy(coefD[:], cps[:, 1:2])
            bps = pcst.tile([P, fo], F32)
            nc.tensor.matmul(out=bps[:], lhsT=ones_row[:], rhs=fcb_sb[:],
                             start=True, stop=True)
            nc.vector.tensor_copy(fcb_bc[:], bps[:])

        # ---- edge phase
        hgat = big.tile([P, ng * fh], F32)
        sum_t = big.tile([P, fh], F32)
        nc.vector.memset(sum_t[:], 0.0)
        ssq_t = big.tile([P, fh], F32)
        nc.vector.memset(ssq_t[:], 0.0)
        dstl_sb = big.tile([P, nt], F32)
        nc.sync.dma_start(dstl_sb[:], dstl)
        mlt_sb = big.tile([P, nt], F32)
        nc.sync.dma_start(mlt_sb[:], mlt)

        with tc.tile_pool(name="xs", bufs=3) as xsp, \
             tc.tile_pool(name="hs", bufs=3) as hsp, \
             tc.tile_pool(name="wp", bufs=3) as wpp, \
             tc.tile_pool(name="ep", bufs=3) as epp, \
             tc.tile_pool(name="fp", bufs=4) as fpp, \
             tc.tile_pool(name="ph", bufs=3, space="PSUM") as php, \
             tc.tile_pool(name="pb", bufs=2, space="PSUM") as pbp:
            for g in range(ng):
                e0 = g * gt * P
                xs = xsp.tile([fi, gt * P], BF16, tag="xs")
                nc.sync.dma_start(xs[:], xsrc[:, e0:e0 + gt * P])
                xd = xsp.tile([fi, gt * P], BF16, tag="xd")
                nc.sync.dma_start(xd[:], xdst[:, e0:e0 + gt * P])
                # pass A: z = a_src + a_dst accumulated in PSUM by the PE
                zp = php.tile([P, gt], F32, tag="zp")
                for t in range(gt):
                    nc.tensor.matmul(
                        out=zp[:, t:t + 1],
                        lhsT=xs[:, t * P:(t + 1) * P],
                        rhs=Wext[:, fh:fh + 1], start=True, stop=False,
                        skip_group_check=True)
                    nc.tensor.matmul(
                        out=zp[:, t:t + 1],
                        lhsT=xd[:, t * P:(t + 1) * P],
                        rhs=coefD[:], start=False, stop=True,
                        skip_group_check=True)
                # e = exp(leakyrelu(z)), batched over the group
                ls = epp.tile([P, gt], F32, tag="ls")
                nc.vector.tensor_scalar(
                    out=ls[:], in0=zp[:], scalar1=NEG_SLOPE, scalar2=None,
                    op0=mybir.AluOpType.mult)
                nc.vector.tensor_tensor(out=ls[:], in0=ls[:], in1=zp[:],
                                        op=mybir.AluOpType.max)
                eg = epp.tile([P, gt], F32, tag="eg")
                nc.scalar.activation(out=eg[:], in_=ls[:],
                                     func=mybir.ActivationFunctionType.Exp)
                # pass B: h recompute + masked scatter-accumulate
                ps = pbp.tile([P, fh + 1], F32)
                for t in range(gt):
                    hp = php.tile([P, fh], F32, tag="hp")
                    nc.tensor.matmul(
                        out=hp[:], lhsT=xs[:, t * P:(t + 1) * P],
                        rhs=Wext[:, 0:fh], start=True, stop=True)
                    hs = hsp.tile([P, fh + 1], BF16)
                    nc.vector.tensor_scalar(
                        out=hs[:, 0:fh], in0=hp[:],
                        scalar1=eg[:, t:t + 1], scalar2=None,
                        op0=mybir.AluOpType.mult)
                    nc.vector.tensor_copy(hs[:, fh:fh + 1], eg[:, t:t + 1])
                    wt = wpp.tile([P, P], BF16)
                    nc.vector.tensor_scalar(
                        out=wt[:], in0=iota_b[:],
                        scalar1=dstl_sb[:, g * gt + t:g * gt + t + 1],
                        scalar2=mlt_sb[:, g * gt + t:g * gt + t + 1],
                        op0=mybir.AluOpType.is_equal,
                        op1=mybir.AluOpType.mult)
                    nc.tensor.matmul(out=ps[:], lhsT=wt[:], rhs=hs[:],
                                     start=(t == 0), stop=(t == gt - 1))
                den = fpp.tile([P, 1], F32, tag="den")
                nc.scalar.activation(out=den[:], in_=ps[:, fh:fh + 1],
                                     func=mybir.ActivationFunctionType.Identity,
                                     bias=eps_den[:])
                rec = fpp.tile([P, 1], F32, tag="rec")
                nc.vector.reciprocal(rec[:], den[:])
                hg = hgat[:, g * fh:(g + 1) * fh]
                nc.vector.tensor_scalar(
                    out=hg, in0=ps[:, 0:fh], scalar1=rec[:], scalar2=None,
                    op0=mybir.AluOpType.mult)
                nc.vector.tensor_tensor(out=sum_t[:], in0=sum_t[:], in1=hg,
                                        op=mybir.AluOpType.add)
                sq = fpp.tile([P, fh], F32, tag="sq")
                nc.vector.tensor_tensor(out=sq[:], in0=hg, in1=hg,
                                        op=mybir.AluOpType.mult)
                nc.vector.tensor_tensor(out=ssq_t[:], in0=ssq_t[:],
                                        in1=sq[:], op=mybir.AluOpType.add)

        # ---- BN stats across cores
        coef = cst.tile([fh, 8], F32)
        with tc.tile_pool(name="psb", bufs=1, space="PSUM") as psb, \
             tc.tile_pool(name="stb", bufs=1) as stb:
            sps = psb.tile([fh, 2], F32)
            nc.tensor.matmul(out=sps[:, 0:1], lhsT=sum_t[:], rhs=ones_col[:],
                             start=True, stop=True)
            nc.tensor.matmul(out=sps[:, 1:2], lhsT=ssq_t[:], rhs=ones_col[:],
                             start=True, stop=True)
            st_sb = stb.tile([fh, 2], F32)
            nc.vector.tensor_copy(st_sb[:], sps[:])
            nc.sync.dma_start(stats_d, st_sb[:])
            nc.gpsimd.collective_compute(
                "AllReduce", mybir.AluOpType.add, replica_groups=rg,
                ins=[stats_d], outs=[stats_d])
            st2 = stb.tile([fh, 2], F32)
            nc.sync.dma_start(st2[:], stats_d)
            nc.vector.tensor_scalar(
                out=coef[:, 0:2], in0=st2[:], scalar1=1.0 / n_nodes,
                scalar2=None, op0=mybir.AluOpType.mult)
            nc.vector.tensor_tensor(out=coef[:, 2:3], in0=coef[:, 0:1],
                                    in1=coef[:, 0:1], op=mybir.AluOpType.mult)
            nc.vector.tensor_tensor(out=coef[:, 3:4], in0=coef[:, 1:2],
                                    in1=coef[:, 2:3],
                                    op=mybir.AluOpType.subtract)
            nc.scalar.activation(out=coef[:, 4:5], in_=coef[:, 3:4],
                                 func=mybir.ActivationFunctionType.Sqrt,
                                 bias=eps_bn[:])
            nc.vector.reciprocal(coef[:, 5:6], coef[:, 4:5])
            nc.vector.tensor_tensor(out=coef[:, 6:7], in0=gam_sb[:],
                                    in1=coef[:, 5:6], op=mybir.AluOpType.mult)
            nc.vector.tensor_tensor(out=coef[:, 7:8], in0=coef[:, 0:1],
                                    in1=coef[:, 6:7], op=mybir.AluOpType.mult)
            nc.vector.tensor_tensor(out=coef[:, 7:8], in0=bet_sb[:],
                                    in1=coef[:, 7:8],
                                    op=mybir.AluOpType.subtract)

        # ---- BN + relu + fc + output rows (all compile-time addressing)
        with tc.tile_pool(name="pt", bufs=2, space="PSUM") as ptp, \
             tc.tile_pool(name="pf", bufs=2, space="PSUM") as pfp, \
             tc.tile_pool(name="lh", bufs=3) as lhp, \
             tc.tile_pool(name="of", bufs=3) as ofp:
            for g in range(ng):
                rows = min(gd, npc - g * gd)
                if rows <= 0:
                    break
                tp = ptp.tile([fh, P], F32)
                nc.tensor.transpose(out=tp[:],
                                    in_=hgat[:, g * fh:(g + 1) * fh],
                                    identity=ident[:])
                lh = lhp.tile([fh, P], F32)
                nc.scalar.activation(out=lh[:], in_=tp[:],
                                     func=mybir.ActivationFunctionType.Relu,
                                     bias=coef[:, 7:8], scale=coef[:, 6:7])
                fps = pfp.tile([P, fo], F32)
                nc.tensor.matmul(out=fps[:], lhsT=lh[:], rhs=fcWT_sb[:],
                                 start=True, stop=True)
                ofs = ofp.tile([P, fo], F32)
                nc.vector.tensor_tensor(out=ofs[:], in0=fps[:],
                                        in1=fcb_bc[:],
                                        op=mybir.AluOpType.add)
                nc.sync.dma_start(out[g * gd:g * gd + rows, :],
                                  ofs[0:rows, :])

    nc.compile()
    return nc


# ------------------------------------------------------------------- kernel
def kernel(x, edge_index, W, att_src, att_dst, conv_b, gamma, beta, fc_W,
           fc_b, _run=None):
    n_cores = 8
    x = np.asarray(x, dtype=np.float32)
    n_nodes, fi = x.shape
    W = np.asarray(W, dtype=np.float32)
    fh = W.shape[0]
    fc_W = np.asarray(fc_W, dtype=np.float32)
    fo = fc_W.shape[0]

    per_core, gd, ng, gt, npc = _host_prep(np.asarray(edge_index), x,
                                           n_nodes, n_cores)
    nc = _build_program(n_cores, gd, ng, gt, npc, fi, fh, fo, n_nodes)

    shared = {
        "Wjk": W,
        "WT": np.ascontiguousarray(W.T),
        "attS": np.asarray(att_src, np.float32).reshape(fh, 1),
        "attD": np.asarray(att_dst, np.float32).reshape(fh, 1),
        "fcWT": np.ascontiguousarray(fc_W.T),
        "fcb": np.asarray(fc_b, np.float32).reshape(1, fo),
        "gam": np.asarray(gamma, np.float32).reshape(fh, 1),
        "bet": np.asarray(beta, np.float32).reshape(fh, 1),
    }
    in_maps = []
    for c in range(n_cores):
        m = dict(shared)
        m.update(per_core[c])
        in_maps.append(m)

    if _run is None:
        from concourse.bass_utils import run_bass_kernel_spmd
        res = run_bass_kernel_spmd(nc, in_maps, core_ids=list(range(n_cores)))
        outs = [r["out"] for r in res.results]
    else:
        outs = _run(nc, in_maps)
    return np.concatenate(outs, axis=0)
